# revision 1
# baseline (speedup 1.0000x reference)
"""Distributed KNN (k-nearest-neighbor classify) on 8 Trainium2 NeuronCores.

Strategy (per sharding hint): shard X_train/y_train along num_train across the
8 cores. Each core computes its [1024, 12500] slab of adjusted scores
    s[t, n] = X[t] . Xtr[n] - 0.5*||Xtr[n]||^2
(monotonically equivalent to negative squared euclidean distance per test row)
via TensorE matmuls (K=128 feature contraction + K=1 PSUM-accumulate fold of
the -0.5*||t||^2 bias), then uses the DVE MAX8/MAX_INDEX sort hardware to pull
the top-8 (value, index) per 500-candidate tile. The 25*8=200 candidates per
test per core are DMA'd out; the host merges 8*200=1600 candidates/test,
takes the global top-k (value desc, index asc — matching jax.lax.top_k tie
semantics), gathers labels and majority-votes (argmax -> smallest label on
ties, matching the reference).
"""
import numpy as np
from contextlib import ExitStack

# Problem geometry (hardcoded per contract).
D = 128          # feature dim = contraction dim = partition dim
T = 1024         # num test points
N_TRAIN = 100000
N_CORES = 8
NS = N_TRAIN // N_CORES   # 12500 train points per core
TILE = 500                # candidates per matmul tile (one PSUM bank, <=512 fp32)
NT = NS // TILE           # 25 tiles per core
NG = T // 128             # 8 test groups of 128 (PSUM partition dim)
NCAND = NT * 8            # 200 candidates kept per test per core
NUM_CLASSES = 10

_CACHE = {}


def _build_program():
    import concourse.tile as tile
    from concourse import bacc, mybir

    F32 = mybir.dt.float32
    U32 = mybir.dt.uint32

    nc = bacc.Bacc("TRN2", target_bir_lowering=False, debug=False,
                   num_devices=N_CORES)
    xT = nc.dram_tensor("xT", [D, T], F32, kind="ExternalInput").ap()
    xtrT = nc.dram_tensor("xtrT", [D, NS], F32, kind="ExternalInput").ap()
    negq = nc.dram_tensor("negq", [1, NS], F32, kind="ExternalInput").ap()
    ones = nc.dram_tensor("ones", [1, D], F32, kind="ExternalInput").ap()
    out_vals = nc.dram_tensor("vals", [T, NCAND], F32, kind="ExternalOutput").ap()
    out_idx = nc.dram_tensor("idx", [T, NCAND], U32, kind="ExternalOutput").ap()

    with tile.TileContext(nc) as tc:
        with ExitStack() as ctx:
            consts = ctx.enter_context(tc.tile_pool(name="consts", bufs=1))
            xT_sb = consts.tile([D, T], F32, name="xT_sb", tag="xT")
            nc.sync.dma_start(xT_sb[:], xT[:])
            ones_sb = consts.tile([1, D], F32, name="ones_sb", tag="ones")
            nc.sync.dma_start(ones_sb[:], ones[:])
            negq_sb = consts.tile([1, NS], F32, name="negq_sb", tag="negq")
            nc.sync.dma_start(negq_sb[:], negq[:])

            xtr_pool = ctx.enter_context(tc.tile_pool(name="xtr", bufs=4))
            psum_pool = ctx.enter_context(
                tc.tile_pool(name="ps", bufs=8, space="PSUM"))
            cand = ctx.enter_context(tc.tile_pool(name="cand", bufs=1))
            vals_sb = [cand.tile([128, NCAND], F32, name=f"v{g}", tag=f"v{g}")
                       for g in range(NG)]
            idx_sb = [cand.tile([128, NCAND], U32, name=f"i{g}", tag=f"i{g}")
                      for g in range(NG)]

            for i in range(NT):
                xtr_t = xtr_pool.tile([D, TILE], F32, name="xtr_t")
                nc.sync.dma_start(xtr_t[:], xtrT[:, i * TILE:(i + 1) * TILE])
                for g in range(NG):
                    ps = psum_pool.tile([128, TILE], F32, name="ps")
                    nc.tensor.matmul(ps[:], xT_sb[:, g * 128:(g + 1) * 128],
                                     xtr_t[:], start=True, stop=False)
                    nc.tensor.matmul(ps[:], ones_sb[:1, :],
                                     negq_sb[:1, i * TILE:(i + 1) * TILE],
                                     start=False, stop=True)
                    vslice = vals_sb[g][:, i * 8:(i + 1) * 8]
                    nc.vector.max(vslice, ps[:])
                    nc.vector.max_index(idx_sb[g][:, i * 8:(i + 1) * 8],
                                        vslice, ps[:])
            for g in range(NG):
                nc.sync.dma_start(out_vals[g * 128:(g + 1) * 128, :], vals_sb[g][:])
                nc.sync.dma_start(out_idx[g * 128:(g + 1) * 128, :], idx_sb[g][:])
    nc.compile()
    return nc


def _get_program():
    if "nc" not in _CACHE:
        _CACHE["nc"] = _build_program()
    return _CACHE["nc"]


def _prep_in_maps(X, X_train):
    xT = np.ascontiguousarray(X.T)
    ones = np.ones((1, D), dtype=np.float32)
    in_maps = []
    for c in range(N_CORES):
        shard = X_train[c * NS:(c + 1) * NS]
        xtrT = np.ascontiguousarray(shard.T)
        q = np.einsum("nd,nd->n", shard.astype(np.float64),
                      shard.astype(np.float64))
        negq = (-0.5 * q).astype(np.float32)[None, :]
        in_maps.append({"xT": xT, "xtrT": xtrT, "negq": negq, "ones": ones})
    return in_maps


def _merge_and_vote(results, y_train, k):
    tile_off = np.repeat(np.arange(NT, dtype=np.int64) * TILE, 8)
    all_vals = np.empty((T, N_CORES * NCAND), dtype=np.float32)
    all_idx = np.empty((T, N_CORES * NCAND), dtype=np.int64)
    for c in range(N_CORES):
        vals = results[c]["vals"]
        idx = results[c]["idx"].astype(np.int64) + tile_off[None, :] + c * NS
        all_vals[:, c * NCAND:(c + 1) * NCAND] = vals
        all_idx[:, c * NCAND:(c + 1) * NCAND] = idx

    # top-k by (value desc, global index asc) — matches lax.top_k on -dists.
    order = np.lexsort((all_idx, -all_vals))[:, :k]
    idx_k = np.take_along_axis(all_idx, order, axis=1)
    labels = y_train[idx_k]                                   # [T, k]
    counts = (labels[:, :, None] == np.arange(NUM_CLASSES)).sum(axis=1)
    return np.argmax(counts, axis=1).astype(np.float32)


def kernel(X, X_train, y_train, k):
    from concourse.bass_utils import run_bass_kernel_spmd

    X = np.asarray(X, dtype=np.float32)
    X_train = np.asarray(X_train, dtype=np.float32)
    y_train = np.asarray(y_train)
    k = int(k)
    assert X.shape == (T, D) and X_train.shape == (N_TRAIN, D)
    assert 1 <= k <= 8

    nc = _get_program()
    in_maps = _prep_in_maps(X, X_train)
    res = run_bass_kernel_spmd(nc, in_maps, core_ids=list(range(N_CORES)))
    return _merge_and_vote(res.results, y_train, k)



# revision 2
# speedup vs baseline: 2.5288x; 2.5288x over previous
"""Distributed KNN (k-nearest-neighbor classify) on 8 Trainium2 NeuronCores.

Strategy (per sharding hint): shard X_train/y_train along num_train across the
8 cores. Each core computes its [1024, 12500] slab of adjusted scores
    s[t, n] = X[t] . Xtr[n] - 0.5*||Xtr[n]||^2
(monotonically equivalent to negative squared euclidean distance per test row)
via TensorE matmuls in fp16 (K=128 feature contraction + K=1 PSUM-accumulate
fold of the -0.5*||t||^2 bias), then uses the DVE MAX8/MAX_INDEX sort hardware
to pull the top-8 (value, index) per 500-candidate tile. The 25*8=200
candidates per test per core are DMA'd out; the host merges 8*200=1600
candidates/test, takes the top-32 by (fp16-accurate) device value, rescores
them exactly in float64, and takes the exact global top-k (value desc, index
asc - matching jax.lax.top_k tie semantics), gathers labels and majority-votes
(argmax -> smallest label on ties, matching the reference).

The fp16 matmul perturbs scores by ~1e-2 absolute; the top-8-per-tile cut has
rank slack (8 kept vs 5 needed) and inter-candidate gaps ~1 unit, so the true
top-5 always survive to the host, where exact rescoring fixes the ordering.
"""
import numpy as np
from contextlib import ExitStack

# Problem geometry (hardcoded per contract).
D = 128          # feature dim = contraction dim = partition dim
T = 1024         # num test points
N_TRAIN = 100000
N_CORES = 8
NS = N_TRAIN // N_CORES   # 12500 train points per core
TILE = 500                # candidates per matmul tile (one PSUM bank, <=512 fp32)
NT = NS // TILE           # 25 tiles per core
NG = T // 128             # 8 test groups of 128 (PSUM partition dim)
NCAND = NT * 8            # 200 candidates kept per test per core
NUM_CLASSES = 10
RESCORE = 32              # host rescores this many top candidates per test

_CACHE = {}


def _build_program():
    import concourse.tile as tile
    from concourse import bacc, mybir

    F16 = mybir.dt.float16
    F32 = mybir.dt.float32
    U32 = mybir.dt.uint32

    nc = bacc.Bacc("TRN2", target_bir_lowering=False, debug=False,
                   num_devices=N_CORES)
    xT = nc.dram_tensor("xT", [D, T], F16, kind="ExternalInput").ap()
    xtrT = nc.dram_tensor("xtrT", [D, NS], F16, kind="ExternalInput").ap()
    negq = nc.dram_tensor("negq", [1, NS], F16, kind="ExternalInput").ap()
    ones = nc.dram_tensor("ones", [1, D], F16, kind="ExternalInput").ap()
    out_vals = nc.dram_tensor("vals", [T, NCAND], F32, kind="ExternalOutput").ap()
    out_idx = nc.dram_tensor("idx", [T, NCAND], U32, kind="ExternalOutput").ap()

    with tile.TileContext(nc) as tc:
        with ExitStack() as ctx:
            consts = ctx.enter_context(tc.tile_pool(name="consts", bufs=1))
            xT_sb = consts.tile([D, T], F16, name="xT_sb", tag="xT")
            nc.sync.dma_start(xT_sb[:], xT[:])
            ones_sb = consts.tile([1, D], F16, name="ones_sb", tag="ones")
            nc.sync.dma_start(ones_sb[:], ones[:])
            negq_sb = consts.tile([1, NS], F16, name="negq_sb", tag="negq")
            nc.sync.dma_start(negq_sb[:], negq[:])

            xtr_pool = ctx.enter_context(tc.tile_pool(name="xtr", bufs=4))
            psum_pool = ctx.enter_context(
                tc.tile_pool(name="ps", bufs=8, space="PSUM"))
            cand = ctx.enter_context(tc.tile_pool(name="cand", bufs=1))
            vals_sb = [cand.tile([128, NCAND], F32, name=f"v{g}", tag=f"v{g}")
                       for g in range(NG)]
            idx_sb = [cand.tile([128, NCAND], U32, name=f"i{g}", tag=f"i{g}")
                      for g in range(NG)]

            for i in range(NT):
                xtr_t = xtr_pool.tile([D, TILE], F16, name="xtr_t")
                nc.sync.dma_start(xtr_t[:], xtrT[:, i * TILE:(i + 1) * TILE])
                for g in range(NG):
                    ps = psum_pool.tile([128, TILE], F32, name="ps")
                    nc.tensor.matmul(ps[:], xT_sb[:, g * 128:(g + 1) * 128],
                                     xtr_t[:], start=True, stop=False)
                    nc.tensor.matmul(ps[:], ones_sb[:1, :],
                                     negq_sb[:1, i * TILE:(i + 1) * TILE],
                                     start=False, stop=True)
                    vslice = vals_sb[g][:, i * 8:(i + 1) * 8]
                    nc.vector.max(vslice, ps[:])
                    nc.vector.max_index(idx_sb[g][:, i * 8:(i + 1) * 8],
                                        vslice, ps[:])
            for g in range(NG):
                nc.sync.dma_start(out_vals[g * 128:(g + 1) * 128, :], vals_sb[g][:])
                nc.sync.dma_start(out_idx[g * 128:(g + 1) * 128, :], idx_sb[g][:])
    nc.compile()
    return nc


def _get_program():
    if "nc" not in _CACHE:
        _CACHE["nc"] = _build_program()
    return _CACHE["nc"]


def _prep_in_maps(X, X_train):
    xT = np.ascontiguousarray(X.T.astype(np.float16))
    ones = np.ones((1, D), dtype=np.float16)
    in_maps = []
    for c in range(N_CORES):
        shard = X_train[c * NS:(c + 1) * NS]
        xtrT = np.ascontiguousarray(shard.T.astype(np.float16))
        q = np.einsum("nd,nd->n", shard.astype(np.float64),
                      shard.astype(np.float64))
        negq = (-0.5 * q).astype(np.float16)[None, :]
        in_maps.append({"xT": xT, "xtrT": xtrT, "negq": negq, "ones": ones})
    return in_maps


def _merge_and_vote(results, X, X_train, y_train, k):
    tile_off = np.repeat(np.arange(NT, dtype=np.int64) * TILE, 8)
    all_vals = np.empty((T, N_CORES * NCAND), dtype=np.float32)
    all_idx = np.empty((T, N_CORES * NCAND), dtype=np.int64)
    for c in range(N_CORES):
        vals = results[c]["vals"]
        idx = results[c]["idx"].astype(np.int64) + tile_off[None, :] + c * NS
        all_vals[:, c * NCAND:(c + 1) * NCAND] = vals
        all_idx[:, c * NCAND:(c + 1) * NCAND] = idx

    # Device values are fp16-matmul-accurate (~1e-2 abs). Take a generous
    # top-RESCORE cut by device value, then rescore those exactly.
    part = np.argpartition(-all_vals, RESCORE, axis=1)[:, :RESCORE]
    idx_r = np.take_along_axis(all_idx, part, axis=1)          # [T, R]
    cand_vecs = X_train[idx_r]                                 # [T, R, D]
    s_exact = np.einsum("td,trd->tr", X.astype(np.float64),
                        cand_vecs.astype(np.float64))
    s_exact -= 0.5 * np.einsum("trd,trd->tr", cand_vecs.astype(np.float64),
                               cand_vecs.astype(np.float64))

    # top-k by (value desc, global index asc) - matches lax.top_k on -dists.
    order = np.lexsort((idx_r, -s_exact), axis=1)[:, :k]
    idx_k = np.take_along_axis(idx_r, order, axis=1)
    labels = y_train[idx_k]                                    # [T, k]
    counts = (labels[:, :, None] == np.arange(NUM_CLASSES)).sum(axis=1)
    return np.argmax(counts, axis=1).astype(np.float32)


def kernel(X, X_train, y_train, k):
    from concourse.bass_utils import run_bass_kernel_spmd

    X = np.asarray(X, dtype=np.float32)
    X_train = np.asarray(X_train, dtype=np.float32)
    y_train = np.asarray(y_train)
    k = int(k)
    assert X.shape == (T, D) and X_train.shape == (N_TRAIN, D)
    assert 1 <= k <= 8

    nc = _get_program()
    in_maps = _prep_in_maps(X, X_train)
    res = run_bass_kernel_spmd(nc, in_maps, core_ids=list(range(N_CORES)))
    return _merge_and_vote(res.results, X, X_train, y_train, k)


# revision 6
# speedup vs baseline: 3.3899x; 1.3405x over previous
"""Distributed KNN (k-nearest-neighbor classify) on 8 Trainium2 NeuronCores.

Sharding: X_train/y_train split along num_train across 8 cores (12500 each).

Per core, per 128-test group g (8 groups):
  TensorE (fp16 inputs): scores s[t,n] = X[t].Xtr[n] - ||Xtr[n]||^2/2 into
    PSUM fp32, 13 two-bank tiles of [128, 1000] (12 full + 1 half).
  Egress + max-tree: ScalarE casts even tiles to fp16 SBUF; DVE tensor_max
    folds odd PSUM tiles against them (L1), then a fp16 TT-max tree (2x mode)
    folds 12 tiles -> 3 r-tiles -> 3 half-folds, giving a [128, 2000] slot
    array U where slot j covers <=8 candidates (preimage).
  DVE MAX8 + FIND_INDEX8 over U give top-8 (value, slot) per test per core.

Host: merges 8 cores x 8 slots per test, takes top SETS by device value,
expands each slot's <=8-candidate preimage, rescores candidates exactly in
float64, takes exact top-k (value desc, index asc - lax.top_k semantics),
gathers labels, majority-votes (argmax -> smallest label on ties).

Exactness: device values are within ~0.15 of true s (fp16 matmul + fp16
casts); inter-candidate gaps at the top-5 boundary are ~1-2, and the slot cut
keeps 8 of 2000 while only ~5 competitors can outrank a true top-5 member, so
the true top-5 always survive to the host rescore, which is exact.
"""
import numpy as np
from contextlib import ExitStack

# Problem geometry (hardcoded per contract).
D = 128          # feature dim = contraction dim
T = 1024         # num test points
N_TRAIN = 100000
N_CORES = 8
NS = N_TRAIN // N_CORES   # 12500 train points per core
NG = T // 128             # 8 test groups of 128 (PSUM partition dim)
BANK = 512                # fp32 elems per PSUM bank (matmul max N)
NTILE = 12                # full [128,1024] psum tiles per group (+1 leftover)
LEFT = NS - NTILE * 2 * BANK   # 212 leftover candidates
NSLOT = 3 * BANK + LEFT   # 1748 selection slots per (test, core)
NKEEP = 8                 # top slots kept per (test, core)
NUM_CLASSES = 10
NSETS = 24                # host expands/rescores this many top sets per test
CONVERT = 2               # quads where ScalarE casts both tiles (DVE fp16 TT)

_CACHE = {}


def _build_program():
    import concourse.tile as tile
    from concourse import bacc, mybir

    F16 = mybir.dt.float16
    F32 = mybir.dt.float32
    U16 = mybir.dt.uint16

    nc = bacc.Bacc("TRN2", target_bir_lowering=False, debug=False,
                   num_devices=N_CORES)
    xT = nc.dram_tensor("xT", [D, T], F16, kind="ExternalInput").ap()
    xtrT = nc.dram_tensor("xtrT", [D, NS], F16, kind="ExternalInput").ap()
    negq = nc.dram_tensor("negq", [1, NS], F16, kind="ExternalInput").ap()
    ones = nc.dram_tensor("ones", [1, D], F16, kind="ExternalInput").ap()
    out_vals = nc.dram_tensor("vals", [128, NG * NKEEP], F16,
                              kind="ExternalOutput").ap()
    out_idx = nc.dram_tensor("idx", [128, NG * NKEEP], U16,
                             kind="ExternalOutput").ap()

    mx = mybir.AluOpType.max

    with tile.TileContext(nc) as tc:
        with ExitStack() as ctx:
            consts = ctx.enter_context(tc.tile_pool(name="consts", bufs=1))
            xT_sb = consts.tile([D, T], F16, name="xT_sb", tag="xT")
            nc.sync.dma_start(xT_sb[:], xT[:])
            ones_sb = consts.tile([1, D], F16, name="ones_sb", tag="ones")
            nc.sync.dma_start(ones_sb[:], ones[:])
            negq_sb = consts.tile([1, NS], F16, name="negq_sb", tag="negq")
            nc.sync.dma_start(negq_sb[:], negq[:])
            xtr_sb = consts.tile([D, NS], F16, name="xtr_sb", tag="xtr")
            # split the big upload so compute can start early
            nc.sync.dma_start(xtr_sb[:, :4000], xtrT[:, :4000])
            nc.sync.dma_start(xtr_sb[:, 4000:8000], xtrT[:, 4000:8000])
            nc.sync.dma_start(xtr_sb[:, 8000:], xtrT[:, 8000:])
            v8_all = consts.tile([128, NG * NKEEP], F16, name="v8a", tag="v8a")
            i8_all = consts.tile([128, NG * NKEEP], U16, name="i8a", tag="i8a")

            psum = ctx.enter_context(tc.tile_pool(name="ps", bufs=4,
                                                  space="PSUM"))
            cpool = ctx.enter_context(tc.tile_pool(name="cp", bufs=8))
            mpool = ctx.enter_context(tc.tile_pool(name="mp", bufs=12))
            rpool = ctx.enter_context(tc.tile_pool(name="rp", bufs=6))
            upool = ctx.enter_context(tc.tile_pool(name="up", bufs=3))

            for g in range(NG):
                lhs = xT_sb[:, g * 128:(g + 1) * 128]
                ptiles = []
                # 12 full tiles + 1 leftover tile of biased scores in PSUM
                for t in range(NTILE + 1):
                    P = psum.tile([128, 2 * BANK], F32, name="P")
                    off = t * 2 * BANK
                    spans = ([(0, BANK), (BANK, 2 * BANK)]
                             if t < NTILE else [(0, LEFT)])
                    for lo, hi in spans:
                        sl = slice(lo, hi)
                        nc.tensor.matmul(P[:, sl], lhs,
                                         xtr_sb[:, off + lo:off + hi],
                                         start=True, stop=False)
                        nc.tensor.matmul(P[:, sl], ones_sb[:1, :],
                                         negq_sb[:1, off + lo:off + hi],
                                         start=False, stop=True)
                    ptiles.append(P)
                    # egress as soon as a quad (2 tiles) is ready
                    if t % 2 == 1:
                        q = t // 2
                        a, b = ptiles[t - 1], ptiles[t]
                        if q < 6 - CONVERT:
                            c = cpool.tile([128, 2 * BANK], F16, name="c")
                            nc.scalar.copy(c[:], a[:])
                            m = mpool.tile([128, 2 * BANK], F16, name="m")
                            nc.vector.tensor_tensor(m[:], b[:], c[:], mx)
                        else:
                            ca = cpool.tile([128, 2 * BANK], F16, name="c")
                            nc.scalar.copy(ca[:], a[:])
                            cb = cpool.tile([128, 2 * BANK], F16, name="c")
                            nc.scalar.copy(cb[:], b[:])
                            m = mpool.tile([128, 2 * BANK], F16, name="m")
                            nc.vector.tensor_tensor(m[:], ca[:], cb[:], mx)
                        ptiles[t - 1] = ptiles[t] = m
                U = upool.tile([128, NSLOT], F16, name="U")
                # leftover tile: ScalarE casts straight into U slots
                nc.scalar.copy(U[:, 3 * BANK:], ptiles[NTILE][:, :LEFT])
                # L2: 6 m-tiles -> 3 r-tiles; L3: fold halves into U
                for j in range(3):
                    r = rpool.tile([128, 2 * BANK], F16, name="r")
                    nc.vector.tensor_tensor(r[:], ptiles[4 * j][:],
                                            ptiles[4 * j + 2][:], mx)
                    nc.vector.tensor_tensor(U[:, j * BANK:(j + 1) * BANK],
                                            r[:, :BANK], r[:, BANK:], mx)
                v8 = v8_all[:, g * NKEEP:(g + 1) * NKEEP]
                nc.vector.max(v8, U[:])
                nc.vector.max_index(i8_all[:, g * NKEEP:(g + 1) * NKEEP],
                                    v8, U[:])
            nc.sync.dma_start(out_vals[:], v8_all[:])
            nc.sync.dma_start(out_idx[:], i8_all[:])
    nc.compile()
    return nc


def _get_program():
    if "nc" not in _CACHE:
        _CACHE["nc"] = _build_program()
    return _CACHE["nc"]


def _prep_in_maps(X, X_train):
    xT = np.ascontiguousarray(X.T.astype(np.float16))
    ones = np.ones((1, D), dtype=np.float16)
    in_maps = []
    for c in range(N_CORES):
        shard = X_train[c * NS:(c + 1) * NS]
        xtrT = np.ascontiguousarray(shard.T.astype(np.float16))
        q = np.einsum("nd,nd->n", shard, shard, dtype=np.float64)
        negq = (-0.5 * q).astype(np.float16)[None, :]
        in_maps.append({"xT": xT, "xtrT": xtrT, "negq": negq, "ones": ones})
    return in_maps


def _slot_preimage():
    """slot j in [0,2000) -> up to 8 within-core candidate ids."""
    pre = np.full((NSLOT, 8), -1, dtype=np.int64)
    j = np.arange(NSLOT)
    blk, pp = j // BANK, j % BANK
    full = blk < 3
    pre[full] = (8 * BANK * blk[full] + pp[full])[:, None] \
        + BANK * np.arange(8)
    pre[~full, 0] = NTILE * 2 * BANK + (j[~full] - 3 * BANK)
    return pre


def _merge_and_vote(results, X, X_train, y_train, k):
    pre = _slot_preimage()                                     # [2000, 8]
    # device layout: vals/idx [128, NG*8]: partition p, col g*8+r is
    # test (g*128+p), rank r.
    all_vals = np.empty((T, N_CORES * NKEEP), dtype=np.float32)
    all_slot = np.empty((T, N_CORES * NKEEP), dtype=np.int64)
    for c in range(N_CORES):
        v = results[c]["vals"].astype(np.float32)              # [128, 64]
        s = results[c]["idx"].astype(np.int64)
        v = v.reshape(128, NG, NKEEP).transpose(1, 0, 2).reshape(T, NKEEP)
        s = s.reshape(128, NG, NKEEP).transpose(1, 0, 2).reshape(T, NKEEP)
        all_vals[:, c * NKEEP:(c + 1) * NKEEP] = v
        all_slot[:, c * NKEEP:(c + 1) * NKEEP] = s + c * (NSLOT + 1000000)

    # take top NSETS sets per test by (fuzzy) device value, expand preimages
    part = np.argpartition(-all_vals, NSETS, axis=1)[:, :NSETS]
    slot_r = np.take_along_axis(all_slot, part, axis=1)        # [T, NSETS]
    core_r = slot_r // (NSLOT + 1000000)
    slot_r = slot_r % (NSLOT + 1000000)
    cands = pre[slot_r]                                        # [T, NSETS, 8]
    valid = cands >= 0
    cands = cands + (core_r * NS)[:, :, None]
    cands[~valid] = 0

    flat = cands.reshape(T, -1)                                # [T, NSETS*8]
    vecs = X_train[flat]                                       # [T, M, D]
    s_ex = np.einsum("td,tmd->tm", X, vecs, dtype=np.float64)
    s_ex -= 0.5 * np.einsum("tmd,tmd->tm", vecs, vecs, dtype=np.float64)
    s_ex[~valid.reshape(T, -1)] = -np.inf
    # dedup (a candidate can appear in two sets? no - sets are disjoint by
    # construction: distinct slots of one core cover disjoint candidates)
    order = np.lexsort((flat, -s_ex), axis=1)[:, :k]
    idx_k = np.take_along_axis(flat, order, axis=1)
    labels = y_train[idx_k]                                    # [T, k]
    counts = (labels[:, :, None] == np.arange(NUM_CLASSES)).sum(axis=1)
    return np.argmax(counts, axis=1).astype(np.float32)


def kernel(X, X_train, y_train, k):
    from concourse.bass_utils import run_bass_kernel_spmd

    X = np.asarray(X, dtype=np.float32)
    X_train = np.asarray(X_train, dtype=np.float32)
    y_train = np.asarray(y_train)
    k = int(k)
    assert X.shape == (T, D) and X_train.shape == (N_TRAIN, D)
    assert 1 <= k <= 8

    nc = _get_program()
    in_maps = _prep_in_maps(X, X_train)
    res = run_bass_kernel_spmd(nc, in_maps, core_ids=list(range(N_CORES)))
    return _merge_and_vote(res.results, X, X_train, y_train, k)


# revision 7
# speedup vs baseline: 3.7119x; 1.0950x over previous
"""Distributed KNN (k-nearest-neighbor classify) on 8 Trainium2 NeuronCores.

Sharding: X_train/y_train split along num_train across 8 cores (12500 each).

Per core, per 128-test group g (8 groups):
  TensorE (fp16 inputs): scores s[t,n] = X[t].Xtr[n] - ||Xtr[n]||^2/2 into
    PSUM fp32, 13 two-bank tiles of [128, 1000] (12 full + 1 half).
  Egress + max-tree: ScalarE casts even tiles to fp16 SBUF; DVE tensor_max
    folds odd PSUM tiles against them (L1), then a fp16 TT-max tree (2x mode)
    folds 12 tiles -> 3 r-tiles -> 3 half-folds, giving a [128, 2000] slot
    array U where slot j covers <=8 candidates (preimage).
  DVE MAX8 + FIND_INDEX8 over U give top-8 (value, slot) per test per core.

Host: merges 8 cores x 8 slots per test, takes top SETS by device value,
expands each slot's <=8-candidate preimage, rescores candidates exactly in
float64, takes exact top-k (value desc, index asc - lax.top_k semantics),
gathers labels, majority-votes (argmax -> smallest label on ties).

Exactness: device values are within ~0.15 of true s (fp16 matmul + fp16
casts); inter-candidate gaps at the top-5 boundary are ~1-2, and the slot cut
keeps 8 of 2000 while only ~5 competitors can outrank a true top-5 member, so
the true top-5 always survive to the host rescore, which is exact.
"""
import numpy as np
from contextlib import ExitStack

# Problem geometry (hardcoded per contract).
D = 128          # feature dim = contraction dim
T = 1024         # num test points
N_TRAIN = 100000
N_CORES = 8
NS = N_TRAIN // N_CORES   # 12500 train points per core
NG = T // 128             # 8 test groups of 128 (PSUM partition dim)
BANK = 512                # fp32 elems per PSUM bank (matmul max N)
NTILE = 12                # full [128,1024] psum tiles per group (+1 leftover)
LEFT = NS - NTILE * 2 * BANK   # 212 leftover candidates
NSLOT = 3 * BANK + LEFT   # 1748 selection slots per (test, core)
NKEEP = 8                 # top slots kept per (test, core)
NUM_CLASSES = 10
NSETS = 24                # host expands/rescores this many top sets per test
CONVERT = 2               # quads where ScalarE casts both tiles (DVE fp16 TT)

_CACHE = {}


def _build_program():
    import concourse.tile as tile
    from concourse import bacc, mybir

    F16 = mybir.dt.float16
    F32 = mybir.dt.float32
    U16 = mybir.dt.uint16

    nc = bacc.Bacc("TRN2", target_bir_lowering=False, debug=False,
                   num_devices=N_CORES)
    xT = nc.dram_tensor("xT", [D, T], F16, kind="ExternalInput").ap()
    xtrT = nc.dram_tensor("xtrT", [D, NS], F16, kind="ExternalInput").ap()
    negq = nc.dram_tensor("negq", [1, NS], F16, kind="ExternalInput").ap()
    ones = nc.dram_tensor("ones", [1, D], F16, kind="ExternalInput").ap()
    out_vals = nc.dram_tensor("vals", [128, NG * NKEEP], F16,
                              kind="ExternalOutput").ap()
    out_idx = nc.dram_tensor("idx", [128, NG * NKEEP], U16,
                             kind="ExternalOutput").ap()

    mx = mybir.AluOpType.max

    with tile.TileContext(nc) as tc:
        with ExitStack() as ctx:
            consts = ctx.enter_context(tc.tile_pool(name="consts", bufs=1))
            xT_sb = consts.tile([D, T], F16, name="xT_sb", tag="xT")
            nc.sync.dma_start(xT_sb[:], xT[:])
            ones_sb = consts.tile([1, D], F16, name="ones_sb", tag="ones")
            nc.sync.dma_start(ones_sb[:], ones[:])
            negq_sb = consts.tile([1, NS], F16, name="negq_sb", tag="negq")
            nc.sync.dma_start(negq_sb[:], negq[:])
            xtr_sb = consts.tile([D, NS], F16, name="xtr_sb", tag="xtr")
            # split the big upload so compute can start early
            nc.sync.dma_start(xtr_sb[:, :4000], xtrT[:, :4000])
            nc.sync.dma_start(xtr_sb[:, 4000:8000], xtrT[:, 4000:8000])
            nc.sync.dma_start(xtr_sb[:, 8000:], xtrT[:, 8000:])
            v8_all = consts.tile([128, NG * NKEEP], F16, name="v8a", tag="v8a")
            i8_all = consts.tile([128, NG * NKEEP], U16, name="i8a", tag="i8a")

            psum = ctx.enter_context(tc.tile_pool(name="ps", bufs=4,
                                                  space="PSUM"))
            cpool = ctx.enter_context(tc.tile_pool(name="cp", bufs=8))
            mpool = ctx.enter_context(tc.tile_pool(name="mp", bufs=12))
            rpool = ctx.enter_context(tc.tile_pool(name="rp", bufs=6))
            upool = ctx.enter_context(tc.tile_pool(name="up", bufs=3))

            for g in range(NG):
                lhs = xT_sb[:, g * 128:(g + 1) * 128]
                ptiles = []
                # 12 full tiles + 1 leftover tile of biased scores in PSUM
                for t in range(NTILE + 1):
                    P = psum.tile([128, 2 * BANK], F32, name="P")
                    off = t * 2 * BANK
                    spans = ([(0, BANK), (BANK, 2 * BANK)]
                             if t < NTILE else [(0, LEFT)])
                    for lo, hi in spans:
                        nc.tensor.matmul(P[:, lo:hi], lhs,
                                         xtr_sb[:, off + lo:off + hi],
                                         start=True, stop=False)
                    for lo, hi in spans:
                        nc.tensor.matmul(P[:, lo:hi], ones_sb[:1, :],
                                         negq_sb[:1, off + lo:off + hi],
                                         start=False, stop=True)
                    ptiles.append(P)
                    # egress as soon as a quad (2 tiles) is ready
                    if t % 2 == 1:
                        q = t // 2
                        a, b = ptiles[t - 1], ptiles[t]
                        if q < 6 - CONVERT:
                            c = cpool.tile([128, 2 * BANK], F16, name="c")
                            nc.scalar.copy(c[:], a[:])
                            m = mpool.tile([128, 2 * BANK], F16, name="m")
                            nc.vector.tensor_tensor(m[:], b[:], c[:], mx)
                        else:
                            ca = cpool.tile([128, 2 * BANK], F16, name="c")
                            nc.scalar.copy(ca[:], a[:])
                            cb = cpool.tile([128, 2 * BANK], F16, name="c")
                            nc.scalar.copy(cb[:], b[:])
                            m = mpool.tile([128, 2 * BANK], F16, name="m")
                            nc.vector.tensor_tensor(m[:], ca[:], cb[:], mx)
                        ptiles[t - 1] = ptiles[t] = m
                U = upool.tile([128, NSLOT], F16, name="U")
                # leftover tile: ScalarE casts straight into U slots
                nc.scalar.copy(U[:, 3 * BANK:], ptiles[NTILE][:, :LEFT])
                # L2: 6 m-tiles -> 3 r-tiles; L3: fold halves into U
                for j in range(3):
                    r = rpool.tile([128, 2 * BANK], F16, name="r")
                    nc.vector.tensor_tensor(r[:], ptiles[4 * j][:],
                                            ptiles[4 * j + 2][:], mx)
                    nc.vector.tensor_tensor(U[:, j * BANK:(j + 1) * BANK],
                                            r[:, :BANK], r[:, BANK:], mx)
                v8 = v8_all[:, g * NKEEP:(g + 1) * NKEEP]
                nc.vector.max(v8, U[:])
                nc.vector.max_index(i8_all[:, g * NKEEP:(g + 1) * NKEEP],
                                    v8, U[:])
            nc.sync.dma_start(out_vals[:], v8_all[:])
            nc.sync.dma_start(out_idx[:], i8_all[:])
    nc.compile()
    return nc


def _get_program():
    if "nc" not in _CACHE:
        _CACHE["nc"] = _build_program()
    return _CACHE["nc"]


def _prep_in_maps(X, X_train):
    xT = np.ascontiguousarray(X.T.astype(np.float16))
    ones = np.ones((1, D), dtype=np.float16)
    in_maps = []
    for c in range(N_CORES):
        shard = X_train[c * NS:(c + 1) * NS]
        xtrT = np.ascontiguousarray(shard.T.astype(np.float16))
        q = np.einsum("nd,nd->n", shard, shard, dtype=np.float64)
        negq = (-0.5 * q).astype(np.float16)[None, :]
        in_maps.append({"xT": xT, "xtrT": xtrT, "negq": negq, "ones": ones})
    return in_maps


def _slot_preimage():
    """slot j in [0,2000) -> up to 8 within-core candidate ids."""
    pre = np.full((NSLOT, 8), -1, dtype=np.int64)
    j = np.arange(NSLOT)
    blk, pp = j // BANK, j % BANK
    full = blk < 3
    pre[full] = (8 * BANK * blk[full] + pp[full])[:, None] \
        + BANK * np.arange(8)
    pre[~full, 0] = NTILE * 2 * BANK + (j[~full] - 3 * BANK)
    return pre


def _merge_and_vote(results, X, X_train, y_train, k):
    pre = _slot_preimage()                                     # [2000, 8]
    # device layout: vals/idx [128, NG*8]: partition p, col g*8+r is
    # test (g*128+p), rank r.
    all_vals = np.empty((T, N_CORES * NKEEP), dtype=np.float32)
    all_slot = np.empty((T, N_CORES * NKEEP), dtype=np.int64)
    for c in range(N_CORES):
        v = results[c]["vals"].astype(np.float32)              # [128, 64]
        s = results[c]["idx"].astype(np.int64)
        v = v.reshape(128, NG, NKEEP).transpose(1, 0, 2).reshape(T, NKEEP)
        s = s.reshape(128, NG, NKEEP).transpose(1, 0, 2).reshape(T, NKEEP)
        all_vals[:, c * NKEEP:(c + 1) * NKEEP] = v
        all_slot[:, c * NKEEP:(c + 1) * NKEEP] = s + c * (NSLOT + 1000000)

    # take top NSETS sets per test by (fuzzy) device value, expand preimages
    part = np.argpartition(-all_vals, NSETS, axis=1)[:, :NSETS]
    slot_r = np.take_along_axis(all_slot, part, axis=1)        # [T, NSETS]
    core_r = slot_r // (NSLOT + 1000000)
    slot_r = slot_r % (NSLOT + 1000000)
    cands = pre[slot_r]                                        # [T, NSETS, 8]
    valid = cands >= 0
    cands = cands + (core_r * NS)[:, :, None]
    cands[~valid] = 0

    flat = cands.reshape(T, -1)                                # [T, NSETS*8]
    vecs = X_train[flat]                                       # [T, M, D]
    s_ex = np.einsum("td,tmd->tm", X, vecs, dtype=np.float64)
    s_ex -= 0.5 * np.einsum("tmd,tmd->tm", vecs, vecs, dtype=np.float64)
    s_ex[~valid.reshape(T, -1)] = -np.inf
    # dedup (a candidate can appear in two sets? no - sets are disjoint by
    # construction: distinct slots of one core cover disjoint candidates)
    order = np.lexsort((flat, -s_ex), axis=1)[:, :k]
    idx_k = np.take_along_axis(flat, order, axis=1)
    labels = y_train[idx_k]                                    # [T, k]
    counts = (labels[:, :, None] == np.arange(NUM_CLASSES)).sum(axis=1)
    return np.argmax(counts, axis=1).astype(np.float32)


def kernel(X, X_train, y_train, k):
    from concourse.bass_utils import run_bass_kernel_spmd

    X = np.asarray(X, dtype=np.float32)
    X_train = np.asarray(X_train, dtype=np.float32)
    y_train = np.asarray(y_train)
    k = int(k)
    assert X.shape == (T, D) and X_train.shape == (N_TRAIN, D)
    assert 1 <= k <= 8

    nc = _get_program()
    in_maps = _prep_in_maps(X, X_train)
    res = run_bass_kernel_spmd(nc, in_maps, core_ids=list(range(N_CORES)))
    return _merge_and_vote(res.results, X, X_train, y_train, k)


# revision 9
# speedup vs baseline: 6.3285x; 1.7049x over previous
"""Distributed KNN (k-nearest-neighbor classify) on 8 Trainium2 NeuronCores.

Sharding: X_train/y_train split along num_train across 8 cores. Candidates
are globally sorted by ||t||^2 and dealt round-robin to cores, then each
4096-candidate block is interleaved on the host so that the device max-tree's
fold preimages are 8 CONSECUTIVE-sorted candidates (a tight norm stratum).

Per core, per 128-test group g (8 groups):
  TensorE (fp16): raw dots d[t,n] = X[t].Xtr[n] into PSUM fp32 (12 two-bank
    [128,1024] tiles + one 212-wide leftover). No per-candidate bias matmul.
  Egress + max-tree: ScalarE casts some PSUM tiles to fp16 SBUF; DVE
    tensor_max folds the others against them (L1), then fp16 TT-max folds
    (2x mode) produce a [128,1748] slot array U of raw-dot maxima, where
    slot j covers 8 consecutive-sorted candidates (preimage).
  Slot bias: U += sbias (broadcast [128,1748], built once per core by K=1
    matmuls from a host row of -max(||t||^2 in slot)/2). Using the slot max
    norm caps each slot value at its true best adjusted score, so no slot is
    spuriously inflated, while true-top slots lose at most the within-slot
    norm spread (~0.1).
  DVE MAX8 + FIND_INDEX8 give top-8 (value, slot) per test per core.

Host: merges 8 cores x 8 slots, takes top NSETS sets by device value, expands
preimages, rescores exactly in float64, exact top-k (value desc, index asc =
lax.top_k semantics), gathers labels, majority vote (ties -> smallest label).
"""
import numpy as np
from contextlib import ExitStack

# Problem geometry (hardcoded per contract).
D = 128          # feature dim = contraction dim
T = 1024         # num test points
N_TRAIN = 100000
N_CORES = 8
NS = N_TRAIN // N_CORES   # 12500 train points per core
NG = T // 128             # 8 test groups of 128 (PSUM partition dim)
BANK = 512                # fp32 elems per PSUM bank (matmul max N)
NTILE = 12                # full [128,1024] psum tiles per group (+1 leftover)
LEFT = NS - NTILE * 2 * BANK   # 212 leftover candidates
NSLOT = 3 * BANK + LEFT   # 1748 selection slots per (test, core)
NKEEP = 8                 # top slots kept per (test, core)
NUM_CLASSES = 10
NSETS = 24                # host expands/rescores this many top sets per test
CONVERT = 3               # quads where ScalarE casts both tiles (DVE fp16 TT)

_CACHE = {}


def _pos_to_sorted():
    """device position p (0..NS) -> within-core sorted rank j."""
    p = np.arange(NS)
    blk, rem = p // 4096, p % 4096
    i, pp = rem // BANK, rem % BANK
    j = np.where(p < NTILE * 2 * BANK, 4096 * blk + 8 * pp + i, p)
    return j


_P2J = _pos_to_sorted()
_J2P = np.empty(NS, dtype=np.int64)
_J2P[_P2J] = np.arange(NS)


def _build_program():
    import concourse.tile as tile
    from concourse import bacc, mybir

    F16 = mybir.dt.float16
    F32 = mybir.dt.float32
    U16 = mybir.dt.uint16

    nc = bacc.Bacc("TRN2", target_bir_lowering=False, debug=False,
                   num_devices=N_CORES)
    xT = nc.dram_tensor("xT", [D, T], F16, kind="ExternalInput").ap()
    xtrT = nc.dram_tensor("xtrT", [D, NS], F16, kind="ExternalInput").ap()
    sbias = nc.dram_tensor("sbias", [1, NSLOT], F16, kind="ExternalInput").ap()
    ones = nc.dram_tensor("ones", [1, D], F16, kind="ExternalInput").ap()
    out_vals = nc.dram_tensor("vals", [128, NG * NKEEP], F16,
                              kind="ExternalOutput").ap()
    out_idx = nc.dram_tensor("idx", [128, NG * NKEEP], U16,
                             kind="ExternalOutput").ap()

    mx = mybir.AluOpType.max
    ad = mybir.AluOpType.add

    with tile.TileContext(nc) as tc:
        with ExitStack() as ctx:
            consts = ctx.enter_context(tc.tile_pool(name="consts", bufs=1))
            xT_sb = consts.tile([D, T], F16, name="xT_sb", tag="xT")
            nc.sync.dma_start(xT_sb[:], xT[:])
            ones_sb = consts.tile([1, D], F16, name="ones_sb", tag="ones")
            nc.sync.dma_start(ones_sb[:], ones[:])
            sb_row = consts.tile([1, NSLOT], F16, name="sb_row", tag="sbr")
            nc.sync.dma_start(sb_row[:], sbias[:])
            xtr_sb = consts.tile([D, NS], F16, name="xtr_sb", tag="xtr")
            # split the big upload so compute can start early
            nc.sync.dma_start(xtr_sb[:, :4096], xtrT[:, :4096])
            nc.sync.dma_start(xtr_sb[:, 4096:8192], xtrT[:, 4096:8192])
            nc.sync.dma_start(xtr_sb[:, 8192:], xtrT[:, 8192:])
            v8_all = consts.tile([128, NG * NKEEP], F16, name="v8a", tag="v8a")
            i8_all = consts.tile([128, NG * NKEEP], U16, name="i8a", tag="i8a")
            bias_bc = consts.tile([128, NSLOT], F16, name="bias_bc", tag="bb")

            psum = ctx.enter_context(tc.tile_pool(name="ps", bufs=4,
                                                  space="PSUM"))
            cpool = ctx.enter_context(tc.tile_pool(name="cp", bufs=8))
            mpool = ctx.enter_context(tc.tile_pool(name="mp", bufs=12))
            rpool = ctx.enter_context(tc.tile_pool(name="rp", bufs=6))
            upool = ctx.enter_context(tc.tile_pool(name="up", bufs=4))

            # broadcast the slot-bias row across partitions: ones^T @ sbias
            Pb = psum.tile([128, 2 * BANK], F32, name="P")
            nc.tensor.matmul(Pb[:, :BANK], ones_sb[:1, :], sb_row[:1, :BANK],
                             start=True, stop=True)
            nc.tensor.matmul(Pb[:, BANK:], ones_sb[:1, :],
                             sb_row[:1, BANK:2 * BANK], start=True, stop=True)
            nc.scalar.copy(bias_bc[:, :2 * BANK], Pb[:])
            Pb2 = psum.tile([128, 2 * BANK], F32, name="P")
            nc.tensor.matmul(Pb2[:, :BANK], ones_sb[:1, :],
                             sb_row[:1, 2 * BANK:3 * BANK],
                             start=True, stop=True)
            nc.tensor.matmul(Pb2[:, BANK:BANK + LEFT], ones_sb[:1, :],
                             sb_row[:1, 3 * BANK:], start=True, stop=True)
            nc.scalar.copy(bias_bc[:, 2 * BANK:], Pb2[:, :BANK + LEFT])

            for g in range(NG):
                lhs = xT_sb[:, g * 128:(g + 1) * 128]
                ptiles = []
                for t in range(NTILE + 1):
                    P = psum.tile([128, 2 * BANK], F32, name="P")
                    off = t * 2 * BANK
                    spans = ([(0, BANK), (BANK, 2 * BANK)]
                             if t < NTILE else [(0, LEFT)])
                    for lo, hi in spans:
                        nc.tensor.matmul(P[:, lo:hi], lhs,
                                         xtr_sb[:, off + lo:off + hi],
                                         start=True, stop=True)
                    ptiles.append(P)
                    # egress as soon as a quad (2 tiles) is ready
                    if t % 2 == 1:
                        q = t // 2
                        a, b = ptiles[t - 1], ptiles[t]
                        if q < 6 - CONVERT:
                            c = cpool.tile([128, 2 * BANK], F16, name="c")
                            nc.scalar.copy(c[:], a[:])
                            m = mpool.tile([128, 2 * BANK], F16, name="m")
                            nc.vector.tensor_tensor(m[:], b[:], c[:], mx)
                        else:
                            ca = cpool.tile([128, 2 * BANK], F16, name="c")
                            nc.scalar.copy(ca[:], a[:])
                            cb = cpool.tile([128, 2 * BANK], F16, name="c")
                            nc.scalar.copy(cb[:], b[:])
                            m = mpool.tile([128, 2 * BANK], F16, name="m")
                            nc.vector.tensor_tensor(m[:], ca[:], cb[:], mx)
                        ptiles[t - 1] = ptiles[t] = m
                U = upool.tile([128, NSLOT], F16, name="U")
                # leftover tile: ScalarE casts straight into U slots
                nc.scalar.copy(U[:, 3 * BANK:], ptiles[NTILE][:, :LEFT])
                # L2: 6 m-tiles -> 3 r-tiles; L3: fold halves into U
                for j in range(3):
                    r = rpool.tile([128, 2 * BANK], F16, name="r")
                    nc.vector.tensor_tensor(r[:], ptiles[4 * j][:],
                                            ptiles[4 * j + 2][:], mx)
                    nc.vector.tensor_tensor(U[:, j * BANK:(j + 1) * BANK],
                                            r[:, :BANK], r[:, BANK:], mx)
                Ub = upool.tile([128, NSLOT], F16, name="Ub")
                nc.vector.tensor_tensor(Ub[:], U[:], bias_bc[:], ad)
                v8 = v8_all[:, g * NKEEP:(g + 1) * NKEEP]
                nc.vector.max(v8, Ub[:])
                nc.vector.max_index(i8_all[:, g * NKEEP:(g + 1) * NKEEP],
                                    v8, Ub[:])
            nc.sync.dma_start(out_vals[:], v8_all[:])
            nc.sync.dma_start(out_idx[:], i8_all[:])
    nc.compile()
    return nc


def _get_program():
    if "nc" not in _CACHE:
        _CACHE["nc"] = _build_program()
    return _CACHE["nc"]


def _prep(X, X_train):
    """Sort by norm, deal round-robin, interleave blocks; build inputs."""
    xT = np.ascontiguousarray(X.T.astype(np.float16))
    ones = np.ones((1, D), dtype=np.float16)
    q = np.einsum("nd,nd->n", X_train, X_train, dtype=np.float64)
    order = np.argsort(q, kind="stable")           # global sorted ranks
    in_maps, orig_of_pos = [], []
    for c in range(N_CORES):
        Oc = order[c::N_CORES]                     # within-core sorted ids
        pos_ids = Oc[_P2J]                         # device position -> id
        xtrT = np.ascontiguousarray(X_train[pos_ids].T.astype(np.float16))
        # slot bias: -max(q of the slot preimage)/2
        qj = q[Oc]                                 # by sorted rank j
        sb = np.empty(NSLOT, dtype=np.float64)
        full = qj[:NTILE * 2 * BANK].reshape(3, BANK, 8)   # [blk, pp, i]
        sb[:3 * BANK] = -0.5 * full.max(axis=2).reshape(-1)
        sb[3 * BANK:] = -0.5 * qj[NTILE * 2 * BANK:]
        in_maps.append({"xT": xT, "xtrT": xtrT, "ones": ones,
                        "sbias": sb.astype(np.float16)[None, :]})
        orig_of_pos.append(pos_ids)
    return in_maps, orig_of_pos


def _prep_in_maps(X, X_train):
    return _prep(X, X_train)[0]


def _slot_preimage():
    """slot -> up to 8 device positions (within-core)."""
    pre = np.full((NSLOT, 8), -1, dtype=np.int64)
    j = np.arange(NSLOT)
    blk, pp = j // BANK, j % BANK
    full = blk < 3
    pre[full] = (8 * BANK * blk[full] + pp[full])[:, None] \
        + BANK * np.arange(8)
    pre[~full, 0] = NTILE * 2 * BANK + (j[~full] - 3 * BANK)
    return pre


_PRE = _slot_preimage()


def _merge_and_vote(results, orig_of_pos, X, X_train, y_train, k):
    all_vals = np.empty((T, N_CORES * NKEEP), dtype=np.float32)
    all_gid = np.empty((T, N_CORES * NKEEP, 8), dtype=np.int64)
    all_ok = np.empty((T, N_CORES * NKEEP, 8), dtype=bool)
    for c in range(N_CORES):
        v = results[c]["vals"].astype(np.float32)              # [128, 64]
        s = results[c]["idx"].astype(np.int64)
        v = v.reshape(128, NG, NKEEP).transpose(1, 0, 2).reshape(T, NKEEP)
        s = s.reshape(128, NG, NKEEP).transpose(1, 0, 2).reshape(T, NKEEP)
        pre = _PRE[s]                                          # [T, NKEEP, 8]
        ok = pre >= 0
        gid = orig_of_pos[c][np.where(ok, pre, 0)]
        sl = slice(c * NKEEP, (c + 1) * NKEEP)
        all_vals[:, sl] = v
        all_gid[:, sl] = gid
        all_ok[:, sl] = ok

    part = np.argpartition(-all_vals, NSETS, axis=1)[:, :NSETS]
    cands = np.take_along_axis(all_gid, part[:, :, None], axis=1)
    valid = np.take_along_axis(all_ok, part[:, :, None], axis=1)
    flat = np.where(valid, cands, 0).reshape(T, -1)            # [T, NSETS*8]
    vecs = X_train[flat]                                       # [T, M, D]
    s_ex = np.einsum("td,tmd->tm", X, vecs, dtype=np.float64)
    s_ex -= 0.5 * np.einsum("tmd,tmd->tm", vecs, vecs, dtype=np.float64)
    s_ex[~valid.reshape(T, -1)] = -np.inf
    order = np.lexsort((flat, -s_ex), axis=1)[:, :k]
    idx_k = np.take_along_axis(flat, order, axis=1)
    labels = y_train[idx_k]                                    # [T, k]
    counts = (labels[:, :, None] == np.arange(NUM_CLASSES)).sum(axis=1)
    return np.argmax(counts, axis=1).astype(np.float32)


def kernel(X, X_train, y_train, k):
    from concourse.bass_utils import run_bass_kernel_spmd

    X = np.asarray(X, dtype=np.float32)
    X_train = np.asarray(X_train, dtype=np.float32)
    y_train = np.asarray(y_train)
    k = int(k)
    assert X.shape == (T, D) and X_train.shape == (N_TRAIN, D)
    assert 1 <= k <= 8

    nc = _get_program()
    in_maps, orig_of_pos = _prep(X, X_train)
    res = run_bass_kernel_spmd(nc, in_maps, core_ids=list(range(N_CORES)))
    return _merge_and_vote(res.results, orig_of_pos, X, X_train, y_train, k)


# revision 12
# speedup vs baseline: 6.8838x; 1.0877x over previous
"""Distributed KNN (k-nearest-neighbor classify) on 8 Trainium2 NeuronCores.

Sharding: X_train/y_train split along num_train across 8 cores. Candidates
are globally sorted by ||t||^2 and dealt round-robin to cores, then each
4096-candidate block is interleaved on the host so that the device max-tree's
fold preimages are 8 CONSECUTIVE-sorted candidates (a tight norm stratum).

Per core, per 128-test group g (8 groups):
  TensorE (fp16): raw dots d[t,n] = X[t].Xtr[n] into PSUM fp32 (12 two-bank
    [128,1024] tiles + one 212-wide leftover). No per-candidate bias matmul.
  Egress + max-tree: ScalarE casts some PSUM tiles to fp16 SBUF; DVE
    tensor_max folds the others against them (L1), then fp16 TT-max folds
    (2x mode) produce a [128,1748] slot array U of raw-dot maxima, where
    slot j covers 8 consecutive-sorted candidates (preimage).
  Slot bias: U += sbias (broadcast [128,1748], built once per core by K=1
    matmuls from a host row of -max(||t||^2 in slot)/2). Using the slot max
    norm caps each slot value at its true best adjusted score, so no slot is
    spuriously inflated, while true-top slots lose at most the within-slot
    norm spread (~0.1).
  DVE MAX8 + FIND_INDEX8 give top-8 (value, slot) per test per core.

Host: merges 8 cores x 8 slots, takes top NSETS sets by device value, expands
preimages, rescores exactly in float64, exact top-k (value desc, index asc =
lax.top_k semantics), gathers labels, majority vote (ties -> smallest label).
"""
import numpy as np
from contextlib import ExitStack

# Problem geometry (hardcoded per contract).
D = 128          # feature dim = contraction dim
T = 1024         # num test points
N_TRAIN = 100000
N_CORES = 8
NS = N_TRAIN // N_CORES   # 12500 train points per core
NG = T // 128             # 8 test groups of 128 (PSUM partition dim)
BANK = 512                # fp32 elems per PSUM bank (matmul max N)
NTILE = 12                # full [128,1024] psum tiles per group (+1 leftover)
LEFT = NS - NTILE * 2 * BANK   # 212 leftover candidates
FB = BANK // 2            # 256 folded slots per block
NSLOT = 3 * FB + LEFT     # 980 selection slots per (test, core)
TAILL = 100               # lowest-norm ranks routed to 1:1 leftover slots
NMID = NTILE * 2 * BANK   # 12288 mid ranks covered by 16-group slots
NKEEP = 8                 # top slots kept per (test, core)
NUM_CLASSES = 10
NSETS = 24                # host expands/rescores this many top sets per test
CONVERT = 3               # quads where ScalarE casts both tiles (DVE fp16 TT)

_CACHE = {}


def _pos_to_sorted():
    """device position p (0..NS) -> within-core sorted rank j.

    Middle ranks [TAILL, TAILL+NMID) fill the folded region so each final
    slot's 16-member preimage is 16 consecutive sorted ranks (a tight norm
    stratum). The extreme norm tails go to the 1:1 leftover slots, where the
    slot bias is exact (no stratum width at all)."""
    p = np.arange(NS)
    blk, rem = p // 4096, p % 4096
    i, pp = rem // BANK, rem % BANK
    jmid = TAILL + 4096 * blk + 16 * (pp % FB) + i + 8 * (pp // FB)
    t = p - NMID
    jtail = np.where(t < TAILL, t, NMID + t)
    return np.where(p < NMID, jmid, jtail)


_P2J = _pos_to_sorted()
_J2P = np.empty(NS, dtype=np.int64)
_J2P[_P2J] = np.arange(NS)


def _build_program():
    import concourse.tile as tile
    from concourse import bacc, mybir

    F16 = mybir.dt.float16
    F32 = mybir.dt.float32
    U16 = mybir.dt.uint16

    nc = bacc.Bacc("TRN2", target_bir_lowering=False, debug=False,
                   num_devices=N_CORES)
    xT = nc.dram_tensor("xT", [D, T], F16, kind="ExternalInput").ap()
    xtrT = nc.dram_tensor("xtrT", [D, NS], F16, kind="ExternalInput").ap()
    sbias = nc.dram_tensor("sbias", [1, NSLOT], F16, kind="ExternalInput").ap()
    ones = nc.dram_tensor("ones", [1, D], F16, kind="ExternalInput").ap()
    out_vals = nc.dram_tensor("vals", [128, NG * NKEEP], F16,
                              kind="ExternalOutput").ap()
    out_idx = nc.dram_tensor("idx", [128, NG * NKEEP], U16,
                             kind="ExternalOutput").ap()

    mx = mybir.AluOpType.max
    ad = mybir.AluOpType.add

    with tile.TileContext(nc) as tc:
        with ExitStack() as ctx:
            consts = ctx.enter_context(tc.tile_pool(name="consts", bufs=1))
            xT_sb = consts.tile([D, T], F16, name="xT_sb", tag="xT")
            nc.sync.dma_start(xT_sb[:], xT[:])
            ones_sb = consts.tile([1, D], F16, name="ones_sb", tag="ones")
            nc.sync.dma_start(ones_sb[:], ones[:])
            sb_row = consts.tile([1, NSLOT], F16, name="sb_row", tag="sbr")
            nc.sync.dma_start(sb_row[:], sbias[:])
            xtr_sb = consts.tile([D, NS], F16, name="xtr_sb", tag="xtr")
            # split the big upload so compute can start early
            nc.sync.dma_start(xtr_sb[:, :4096], xtrT[:, :4096])
            nc.sync.dma_start(xtr_sb[:, 4096:8192], xtrT[:, 4096:8192])
            nc.sync.dma_start(xtr_sb[:, 8192:], xtrT[:, 8192:])
            v8_all = consts.tile([128, NG * NKEEP], F16, name="v8a", tag="v8a")
            i8_all = consts.tile([128, NG * NKEEP], U16, name="i8a", tag="i8a")
            bias_bc = consts.tile([128, NSLOT], F16, name="bias_bc", tag="bb")

            psum = ctx.enter_context(tc.tile_pool(name="ps", bufs=4,
                                                  space="PSUM"))
            cpool = ctx.enter_context(tc.tile_pool(name="cp", bufs=8))
            mpool = ctx.enter_context(tc.tile_pool(name="mp", bufs=12))
            rpool = ctx.enter_context(tc.tile_pool(name="rp", bufs=6))
            upool = ctx.enter_context(tc.tile_pool(name="up", bufs=4))

            # broadcast the slot-bias row across partitions: ones^T @ sbias
            Pb = psum.tile([128, 2 * BANK], F32, name="P")
            nc.tensor.matmul(Pb[:, :BANK], ones_sb[:1, :], sb_row[:1, :BANK],
                             start=True, stop=True)
            nc.tensor.matmul(Pb[:, BANK:NSLOT], ones_sb[:1, :],
                             sb_row[:1, BANK:], start=True, stop=True)
            nc.scalar.copy(bias_bc[:], Pb[:, :NSLOT])

            for g in range(NG):
                lhs = xT_sb[:, g * 128:(g + 1) * 128]
                ptiles = []
                for t in range(NTILE + 1):
                    P = psum.tile([128, 2 * BANK], F32, name="P")
                    off = t * 2 * BANK
                    spans = ([(0, BANK), (BANK, 2 * BANK)]
                             if t < NTILE else [(0, LEFT)])
                    for lo, hi in spans:
                        nc.tensor.matmul(P[:, lo:hi], lhs,
                                         xtr_sb[:, off + lo:off + hi],
                                         start=True, stop=True)
                    ptiles.append(P)
                    # egress as soon as a quad (2 tiles) is ready
                    if t % 2 == 1:
                        q = t // 2
                        a, b = ptiles[t - 1], ptiles[t]
                        if q < 6 - CONVERT:
                            c = cpool.tile([128, 2 * BANK], F16, name="c")
                            nc.scalar.copy(c[:], a[:])
                            m = mpool.tile([128, 2 * BANK], F16, name="m")
                            nc.vector.tensor_tensor(m[:], b[:], c[:], mx)
                        else:
                            ca = cpool.tile([128, 2 * BANK], F16, name="c")
                            nc.scalar.copy(ca[:], a[:])
                            cb = cpool.tile([128, 2 * BANK], F16, name="c")
                            nc.scalar.copy(cb[:], b[:])
                            m = mpool.tile([128, 2 * BANK], F16, name="m")
                            nc.vector.tensor_tensor(m[:], ca[:], cb[:], mx)
                        ptiles[t - 1] = ptiles[t] = m
                U = upool.tile([128, 3 * BANK], F16, name="U")
                V = upool.tile([128, NSLOT], F16, name="V")
                # leftover tile: ScalarE casts straight into V slots
                nc.scalar.copy(V[:, 3 * FB:], ptiles[NTILE][:, :LEFT])
                # L2: 6 m-tiles -> 3 r-tiles; L3/L4: fold into U then V
                for j in range(3):
                    r = rpool.tile([128, 2 * BANK], F16, name="r")
                    nc.vector.tensor_tensor(r[:], ptiles[4 * j][:],
                                            ptiles[4 * j + 2][:], mx)
                    nc.vector.tensor_tensor(U[:, j * BANK:(j + 1) * BANK],
                                            r[:, :BANK], r[:, BANK:], mx)
                    nc.vector.tensor_tensor(
                        V[:, j * FB:(j + 1) * FB],
                        U[:, j * BANK:j * BANK + FB],
                        U[:, j * BANK + FB:(j + 1) * BANK], mx)
                Ub = upool.tile([128, NSLOT], F16, name="Ub")
                nc.vector.tensor_tensor(Ub[:], V[:], bias_bc[:], ad)
                v8 = v8_all[:, g * NKEEP:(g + 1) * NKEEP]
                nc.vector.max(v8, Ub[:])
                nc.vector.max_index(i8_all[:, g * NKEEP:(g + 1) * NKEEP],
                                    v8, Ub[:])
            nc.sync.dma_start(out_vals[:], v8_all[:])
            nc.sync.dma_start(out_idx[:], i8_all[:])
    nc.compile()
    return nc


def _get_program():
    if "nc" not in _CACHE:
        _CACHE["nc"] = _build_program()
    return _CACHE["nc"]


def _prep(X, X_train):
    """Sort by norm, deal round-robin, interleave blocks; build inputs."""
    xT = np.ascontiguousarray(X.T.astype(np.float16))
    ones = np.ones((1, D), dtype=np.float16)
    q = np.einsum("nd,nd->n", X_train, X_train, dtype=np.float64)
    order = np.argsort(q, kind="stable")           # global sorted ranks
    in_maps, sorted_ids = [], []
    for c in range(N_CORES):
        Oc = order[c::N_CORES]                     # within-core sorted ids
        pos_ids = Oc[_P2J]                         # device position -> id
        xtrT = np.ascontiguousarray(X_train[pos_ids].T.astype(np.float16))
        # slot bias: -max(q of the slot preimage)/2
        qj = q[Oc]                                 # by sorted rank j
        sb = np.empty(NSLOT, dtype=np.float64)
        full = qj[TAILL:TAILL + NMID].reshape(3, FB, 16)   # [blk, pp2, m]
        sb[:3 * FB] = -0.5 * full.max(axis=2).reshape(-1)
        tl = np.arange(LEFT)
        sb[3 * FB:] = -0.5 * qj[np.where(tl < TAILL, tl, NMID + tl)]
        in_maps.append({"xT": xT, "xtrT": xtrT, "ones": ones,
                        "sbias": sb.astype(np.float16)[None, :]})
        sorted_ids.append(Oc)
    return in_maps, sorted_ids


def _prep_in_maps(X, X_train):
    return _prep(X, X_train)[0]


def _slot_preimage():
    """slot -> up to 16 within-core SORTED RANKS (consecutive)."""
    pre = np.full((NSLOT, 16), -1, dtype=np.int64)
    s = np.arange(NSLOT)
    blk, pp2 = s // FB, s % FB
    full = blk < 3
    pre[full] = (TAILL + 4096 * blk[full] + 16 * pp2[full])[:, None] \
        + np.arange(16)
    t = s[~full] - 3 * FB
    pre[~full, 0] = np.where(t < TAILL, t, NMID + t)
    return pre


_PRE = _slot_preimage()


def _merge_and_vote(results, sorted_ids, X, X_train, y_train, k):
    all_vals = np.empty((T, N_CORES * NKEEP), dtype=np.float32)
    all_gid = np.empty((T, N_CORES * NKEEP, 16), dtype=np.int64)
    all_ok = np.empty((T, N_CORES * NKEEP, 16), dtype=bool)
    for c in range(N_CORES):
        v = results[c]["vals"].astype(np.float32)              # [128, 64]
        s = results[c]["idx"].astype(np.int64)
        v = v.reshape(128, NG, NKEEP).transpose(1, 0, 2).reshape(T, NKEEP)
        s = s.reshape(128, NG, NKEEP).transpose(1, 0, 2).reshape(T, NKEEP)
        pre = _PRE[s]                                          # [T, NKEEP, 16]
        ok = pre >= 0
        gid = sorted_ids[c][np.where(ok, pre, 0)]
        sl = slice(c * NKEEP, (c + 1) * NKEEP)
        all_vals[:, sl] = v
        all_gid[:, sl] = gid
        all_ok[:, sl] = ok

    def rescore(sets):
        cands = np.take_along_axis(all_gid, sets[:, :, None], axis=1)
        valid = np.take_along_axis(all_ok, sets[:, :, None], axis=1)
        flat = np.where(valid, cands, 0).reshape(T, -1)
        vecs = X_train[flat].astype(np.float64)                # [T, M, D]
        s = np.matmul(vecs, X.astype(np.float64)[:, :, None])[:, :, 0]
        s -= 0.5 * np.einsum("tmd,tmd->tm", vecs, vecs)
        s[~valid.reshape(T, -1)] = -np.inf
        return flat, s

    srt = np.argsort(-all_vals, axis=1)                        # [T, 64]
    flat1, s1 = rescore(srt[:, :8])
    kth = -np.sort(-s1, axis=1)[:, k - 1]                      # provisional
    vals_s = np.take_along_axis(all_vals, srt, axis=1)
    need = vals_s[:, 8:] >= (kth[:, None] - 0.25)              # upper bounds
    nmore = int(need.sum(axis=1).max())
    if nmore > 0:
        flat2, s2 = rescore(srt[:, 8:8 + nmore])
        s2[~need[:, :nmore]] = -np.inf
        flat1 = np.concatenate([flat1, flat2], axis=1)
        s1 = np.concatenate([s1, s2], axis=1)
    order = np.lexsort((flat1, -s1), axis=1)[:, :k]
    idx_k = np.take_along_axis(flat1, order, axis=1)
    labels = y_train[idx_k]                                    # [T, k]
    counts = (labels[:, :, None] == np.arange(NUM_CLASSES)).sum(axis=1)
    return np.argmax(counts, axis=1).astype(np.float32)


def kernel(X, X_train, y_train, k):
    from concourse.bass_utils import run_bass_kernel_spmd

    X = np.asarray(X, dtype=np.float32)
    X_train = np.asarray(X_train, dtype=np.float32)
    y_train = np.asarray(y_train)
    k = int(k)
    assert X.shape == (T, D) and X_train.shape == (N_TRAIN, D)
    assert 1 <= k <= 8

    nc = _get_program()
    in_maps, sorted_ids = _prep(X, X_train)
    res = run_bass_kernel_spmd(nc, in_maps, core_ids=list(range(N_CORES)))
    return _merge_and_vote(res.results, sorted_ids, X, X_train, y_train, k)
